# revision 28
# baseline (speedup 1.0000x reference)
"""Causal multi-head attention (B=1, N=4096, H=16, D=64) on 8 trn2 cores.

Head-parallel tensor parallelism: 2 heads per core.  Each core reads the
full x (pre-transposed on host), computes QKV for its 2 heads, runs causal
attention in the S^T (k-partition, q-free) layout, applies its 128-column
slice of the output projection, and writes a partial [4096, 1024] y that
the host sums (plus b_proj).

Single fused pipeline: the Q/K projection of chunk j+1, the V' projection
of chunk j, and the output projection of q-block j-1 are interleaved into
the attention kt-loop of q-block j as background PE work so the tensor
engine never idles (keeping its p-state at max).  Causal masking is
multiplicative on P after exp (off the S->exp critical path), diagonal
tiles are narrowed to their valid column range, V' is produced directly by
the V projection in [seq, d] layout (no transposes), 1/l is broadcast
across partitions via a small DMA + gpsimd partition_broadcast (mid
blocks) or a K=1 PE matmul + head-split projection (final block, keeping
the tail on-chip), x/weight loads are split across engines and queues with
chunk loads deferred until one block before use, and the y writes stream
out per 128x512 tile from the idle gpsimd engine.
"""

from collections import deque

import numpy as np

import concourse.bass as bass
from concourse import bacc, masks
import concourse.tile as tile
from concourse import mybir
from concourse.bass_utils import run_bass_kernel_spmd

B, N, H, D = 1, 4096, 16, 64
C = H * D  # 1024
SCALE = D ** -0.5
NCORES = 8
HPC = H // NCORES  # heads per core = 2
F32 = mybir.dt.float32
BF16 = mybir.dt.bfloat16

NKT = N // 128      # 32 k-tiles of 128
NQB = N // 512      # 8 q-blocks of 512
NCT = C // 128      # 8 contraction tiles for the projections


DEBUG_TAPS = False


def build_nc():
    nc = bacc.Bacc("TRN2", target_bir_lowering=False)

    # xb[j, p, ct, :]: chunk-major, per-partition contiguous (8KB descriptors)
    xb = nc.dram_tensor("xb", [NQB, 128, NCT, 512], BF16,
                        kind="ExternalInput").ap()
    wqk = nc.dram_tensor("wqk", [128, NCT, 256], BF16,
                         kind="ExternalInput").ap()
    wv = nc.dram_tensor("wv", [128, NCT, 128], BF16,
                        kind="ExternalInput").ap()
    wp = nc.dram_tensor("wp", [128, C], BF16, kind="ExternalInput").ap()
    tri = nc.dram_tensor("tri", [128, 128], BF16, kind="ExternalInput").ap()
    y = nc.dram_tensor("y", [N, C], BF16, kind="ExternalOutput").ap()

    taps = None
    if DEBUG_TAPS:
        taps = {
            "dL": nc.dram_tensor("dL", [NQB, 2, 512], F32,
                                 kind="ExternalOutput").ap(),
            "dLinv": nc.dram_tensor("dLinv", [NQB, 2, 512], F32,
                                    kind="ExternalOutput").ap(),
            "dOn": nc.dram_tensor("dOn", [NQB, 128, 512], BF16,
                                  kind="ExternalOutput").ap(),
            "dQT": nc.dram_tensor("dQT", [128, N], BF16,
                                  kind="ExternalOutput").ap(),
            "dKT": nc.dram_tensor("dKT", [128, N], BF16,
                                  kind="ExternalOutput").ap(),
            "dVP": nc.dram_tensor("dVP", [128, NKT * 130], BF16,
                                  kind="ExternalOutput").ap(),
        }

    with tile.TileContext(nc) as tc:
        _body(tc, xb, wqk, wv, wp, tri, y, taps)
    nc.compile()
    return nc


def _body(tc, xb, wqk, wv, wp, tri, y, taps=None):
    nc = tc.nc
    Exp = mybir.ActivationFunctionType.Exp
    Copy = mybir.ActivationFunctionType.Copy

    persist = tc.alloc_tile_pool(name="persist", bufs=1)

    # Persistent SBUF tensors
    xt = persist.tile([128, NCT, N], BF16, tag="xt")      # full x^T, resident
    QT = persist.tile([128, N], BF16, tag="QT")           # [(h,d), n]
    KT = persist.tile([128, N], BF16, tag="KT")
    # V' per kt tile: [k, (h0 d0..63, ones, h1 d0..63, ones)] so each head's
    # lhsT slice [k, 65] is contiguous and the DMA-transposed d-block lands
    # in a contiguous destination (xbar transpose needs contiguous dst).
    VP = persist.tile([128, NKT, 130], BF16, tag="VP")
    Wqk = persist.tile([128, NCT, 256], BF16, tag="Wqk")
    Wv = persist.tile([128, NCT, 128], BF16, tag="Wv")
    Wp = persist.tile([128, C], BF16, tag="Wp")
    WpB = persist.tile([64, C], BF16, tag="WpB")          # Wp rows 64..127
    tri_sb = persist.tile([128, 128], BF16, tag="tri")    # [kp, qc] 1 if qc>=kp
    ones65 = persist.tile([65, 64], F32, tag="ones65")    # row 64 used as lhsT

    # startup: a single dma_start's descriptors land on ONE queue, so split
    # the critical chunk-0 / Wqk loads into several triggers spread across
    # engines (parallel queues).  Later chunks are issued an iteration ahead.
    def load_chunk(j, eng, nsplit=2):
        w = NCT // nsplit
        for i in range(nsplit):
            eng.dma_start(
                out=xt[:, w * i : w * (i + 1), 512 * j : 512 * (j + 1)],
                in_=xb[j, :, w * i : w * (i + 1), :])

    # the first trigger on each engine dispatches fastest: make them the two
    # pieces the very first matmul needs (x chunk-0 ct0 and Wqk ct0-1)
    nc.sync.dma_start(out=xt[:, 0:1, 0:512], in_=xb[0, :, 0:1, :])
    nc.scalar.dma_start(out=Wqk[:, 0:2, :], in_=wqk[:, 0:2, :])
    nc.sync.dma_start(out=Wqk[:, 2:5, :], in_=wqk[:, 2:5, :])
    nc.scalar.dma_start(out=xt[:, 1:2, 0:512], in_=xb[0, :, 1:2, :])
    nc.sync.dma_start(out=xt[:, 2:3, 0:512], in_=xb[0, :, 2:3, :])
    nc.scalar.dma_start(out=Wqk[:, 5:8, :], in_=wqk[:, 5:8, :])
    nc.scalar.dma_start(out=xt[:, 3:4, 0:512], in_=xb[0, :, 3:4, :])
    for i in range(4, 8):
        nc.gpsimd.dma_start(out=xt[:, i : i + 1, 0:512],
                            in_=xb[0, :, i : i + 1, :])
    nc.sync.dma_start(out=Wv[:, 0:4, :], in_=wv[:, 0:4, :])
    nc.scalar.dma_start(out=Wv[:, 4:8, :], in_=wv[:, 4:8, :])
    nc.sync.dma_start(out=tri_sb, in_=tri)
    load_chunk(1, nc.gpsimd, nsplit=2)
    nc.gpsimd.dma_start(out=Wp, in_=wp)
    nc.gpsimd.dma_start(out=WpB, in_=wp[64:128, :])
    nc.vector.memset(ones65, 1.0)

    # ones columns of V' (indices 64 and 129 of the last axis)
    ones_st = persist.tile([128, NKT * 2], F32, tag="ones_st")
    nc.vector.memset(ones_st, 1.0)
    nc.vector.tensor_copy(
        VP.rearrange("p t (g c) -> p (t g) c", g=2)[:, :, 64:65],
        ones_st.rearrange("p (n o) -> p n o", o=1),
    )

    with (
        tc.tile_pool(name="spool", bufs=2, space="PSUM") as spool,   # 4 banks
        tc.tile_pool(name="opool", bufs=1, space="PSUM") as opool,   # 2 banks
        tc.tile_pool(name="smpool", bufs=2, space="PSUM") as smpool, # 2 banks
        tc.tile_pool(name="ptpool", bufs=3) as ptpool,
        tc.tile_pool(name="sbpool", bufs=2) as sbpool,
        tc.tile_pool(name="ybpool", bufs=2) as ybpool,
        tc.tile_pool(name="vtpool", bufs=2) as vtpool,
        tc.tile_pool(name="stgpool", bufs=3) as stgpool,
    ):
        def qk_closure(j):
            """Q/K projection for sequence chunk j (out [och, seq]).  Q and K
            are interleaved so consecutive matmuls accumulate into different
            PSUM banks (avoids same-bank read-modify-write bubbles)."""
            cls = []
            holder = {}

            def qk_piece(c0):
                def run():
                    if c0 == 0:
                        holder["q"] = smpool.tile([128, 512], F32, tag="sm",
                                                  name=f"psq_{j}")
                        holder["k"] = smpool.tile([128, 512], F32, tag="sm",
                                                  name=f"psk_{j}")
                    psq, psk = holder["q"], holder["k"]
                    for ct in range(c0, c0 + 2):
                        nc.tensor.matmul(
                            psq, Wqk[:, ct, 0:128],
                            xt[:, ct, 512 * j : 512 * (j + 1)],
                            start=(ct == 0), stop=(ct == NCT - 1),
                        )
                        nc.tensor.matmul(
                            psk, Wqk[:, ct, 128:256],
                            xt[:, ct, 512 * j : 512 * (j + 1)],
                            start=(ct == 0), stop=(ct == NCT - 1),
                        )
                    if c0 == NCT - 2:
                        nc.vector.tensor_copy(QT[:, 512 * j : 512 * (j + 1)],
                                              psq)
                        nc.vector.tensor_copy(KT[:, 512 * j : 512 * (j + 1)],
                                              psk)
                return run

            for c0 in range(0, NCT, 2):
                cls.append(qk_piece(c0))
            return cls

        def v_closures(j):
            """V for chunk j, computed as V^T [(h d), seq] with N=512
            matmuls (4x fewer PE issues than the [seq, d] form), then
            rotated into the V' [seq, d] layout by the DMA xbar transpose
            (sync engine; dst offset must be 0, hence the staging tile)
            plus a cheap 4x-mode DVE copy."""
            cls = []
            holder = {}

            def vt_piece(c0):
                def run():
                    if c0 == 0:
                        holder["vt"] = smpool.tile([128, 512], F32, tag="sm",
                                                   name=f"vtps_{j}")
                    ps = holder["vt"]
                    for ct in range(c0, c0 + 2):
                        nc.tensor.matmul(
                            ps, Wv[:, ct, :],
                            xt[:, ct, 512 * j : 512 * (j + 1)],
                            start=(ct == 0), stop=(ct == NCT - 1),
                        )
                    if c0 == NCT - 2:
                        vts = vtpool.tile([128, 512], BF16, tag="vts",
                                          name=f"vts_{j}")
                        nc.vector.tensor_copy(vts, ps)
                        for i in range(4):
                            kt = 4 * j + i
                            stg = stgpool.tile([128, 128], BF16, tag="stg",
                                               name=f"stg_{j}_{i}")
                            nc.sync.dma_start_transpose(
                                stg, vts[:, 128 * i : 128 * (i + 1)])
                            nc.vector.tensor_copy(
                                VP[:, kt, :].rearrange(
                                    "p (g c) -> p g c", g=2)[:, :, 0:64],
                                stg.rearrange("p (g c) -> p g c", g=2),
                            )
                return run

            for c0 in range(0, NCT, 2):
                cls.append(vt_piece(c0))
            return cls

        def proj_closures(qb, eps):
            """Normalize o by 1/l (PE partition-broadcast) and project."""
            cls = []
            onorm = sbpool.tile([128, 512], BF16, tag="onorm",
                                name=f"onorm_{qb}")

            def norm_mul(h):
                def run():
                    if h == 0:
                        nc.vector.tensor_mul(onorm[0:64, :],
                                             eps["osb"][0:64, 0, :],
                                             eps["linv"][:, 0, :])
                    else:
                        tmp = sbpool.tile([64, 512], BF16, tag="tmp",
                                          name=f"tmp_{qb}")
                        nc.vector.tensor_mul(tmp, eps["osb"][0:64, 1, :],
                                             eps["linv"][:, 1, :])
                        nc.sync.dma_start(out=onorm[64:128, :], in_=tmp)
                        if taps is not None:
                            nc.gpsimd.dma_start(out=taps["dL"][qb],
                                                in_=eps["osb"][64:65, :, :])
                            nc.gpsimd.dma_start(out=taps["dLinv"][qb],
                                                in_=eps["linv"][0:1, :, :])
                            nc.gpsimd.dma_start(out=taps["dOn"][qb], in_=onorm)
                return run

            cls.append(norm_mul(0))
            cls.append(norm_mul(1))

            ybuf = ybpool.tile([128, 4, C], BF16, tag="ybuf", name=f"yb_{qb}")

            def proj(s, oc):
                def run():
                    yps = smpool.tile([128, 512], F32, tag="sm")
                    nc.tensor.matmul(
                        yps, onorm[:, 128 * s : 128 * (s + 1)],
                        Wp[:, 512 * oc : 512 * (oc + 1)],
                        start=True, stop=True,
                    )
                    nc.vector.tensor_copy(ybuf[:, s, 512 * oc : 512 * (oc + 1)],
                                          yps)
                    q0 = 512 * qb + 128 * s
                    nc.gpsimd.dma_start(
                        out=y[q0 : q0 + 128, 512 * oc : 512 * (oc + 1)],
                        in_=ybuf[:, s, 512 * oc : 512 * (oc + 1)])
                return run

            for s in range(4):
                for oc in range(2):
                    cls.append(proj(s, oc))
            return cls

        # ---- fused main loop ----
        bg = deque(qk_closure(0) + v_closures(0) + v_closures(1))
        while bg:
            bg.popleft()()

        eps_prev = None
        for qb in range(NQB):
            n_kt = 4 * (qb + 1)
            q0 = 512 * qb

            if qb + 2 < NQB:
                load_chunk(qb + 2, nc.gpsimd)

            # background PE work for this block's kt loop: Q/K of the next
            # chunk (needed right at the next block's start), V' TWO chunks
            # ahead (its matmul -> evac -> xbar-transpose chain is long, so
            # give it a full block of slack), and the previous block's
            # projection.
            bg = deque()
            if qb + 1 < NQB:
                bg.extend(qk_closure(qb + 1))
            if qb + 2 < NQB:
                bg.extend(v_closures(qb + 2))
            if eps_prev is not None:
                bg.extend(proj_closures(qb - 1, eps_prev))

            o_ps = opool.tile([65, 2, 512], F32, tag="o", name=f"o_{qb}")
            s_tiles = {}

            def emit_S(kt, qb=qb, q0=q0, s_tiles=s_tiles):
                jr = kt - 4 * qb
                qs = 128 * jr if jr >= 0 else 0
                s_ps = spool.tile([128, 2, 512], F32, tag="s",
                                  name=f"s_{qb}_{kt}")
                for h in range(2):
                    nc.tensor.matmul(
                        s_ps[:, h, qs:512],
                        KT[64 * h : 64 * h + 64, 128 * kt : 128 * (kt + 1)],
                        QT[64 * h : 64 * h + 64, q0 + qs : q0 + 512],
                        start=True, stop=True,
                    )
                s_tiles[kt] = (s_ps, qs)

            emit_S(0)
            bg0 = len(bg)
            drained = 0
            for kt in range(n_kt):
                if kt + 1 < n_kt:
                    emit_S(kt + 1)
                s_ps, qs = s_tiles.pop(kt)
                pt = ptpool.tile([128, 2, 512], BF16, tag="pt",
                                 name=f"pt_{qb}_{kt}")
                nc.scalar.activation(pt[:, :, qs:512], s_ps[:, :, qs:512],
                                     Exp, scale=SCALE)
                if kt - 4 * qb >= 0:  # diagonal tile: zero invalid triangle
                    # one op for both heads; tri broadcast over the h dim
                    # via a stride-0 middle AP dim
                    tri_b = bass.AP(tensor=tri_sb.tensor, offset=tri_sb.offset,
                                    ap=[list(tri_sb.ap[0]), [0, 2],
                                        list(tri_sb.ap[1])])
                    nc.vector.tensor_mul(pt[:, :, qs : qs + 128],
                                         pt[:, :, qs : qs + 128], tri_b)
                for h in range(2):
                    nc.tensor.matmul(
                        o_ps[:, h, qs:512], VP[:, kt, 65 * h : 65 * h + 65],
                        pt[:, h, qs:512],
                        start=(kt == 0), stop=(kt == n_kt - 1),
                    )
                # drain background PE work spread uniformly over ALL
                # iterations (ceil-of-remaining exhausts the queue early and
                # starves the tail iterations of filler work)
                target = -(-(kt + 1) * bg0 // n_kt)
                while drained < target and bg:
                    bg.popleft()()
                    drained += 1

            # epilogue: free o_ps via vector copy (scalar is the exp
            # bottleneck engine; gpsimd cannot read PSUM).  For the last
            # block, split so the l row (scalar) lands in parallel with the
            # d rows (vector) -- shortens the tail's serial chain.
            osb = sbpool.tile([65, 2, 512], F32, tag="osb", name=f"osb_{qb}")
            if qb == NQB - 1:
                nc.scalar.activation(osb[64:65, :, :], o_ps[64:65, :, :],
                                     Copy)
                nc.vector.tensor_copy(osb[0:64, :, :], o_ps[0:64, :, :])
            else:
                nc.vector.tensor_copy(osb, o_ps)
            if qb < NQB - 1:
                # move the l row to partition 0 (small DMA), 1/l there, then
                # gpsimd partition_broadcast across partitions 0..63 (DVE
                # lanes cannot cross partitions / need quad-aligned bases)
                # gpsimd, not sync: the sync queue carries the xbar
                # transposes, which must not delay this latency-critical hop
                l0 = sbpool.tile([1, 2, 512], F32, tag="l0", name=f"l0_{qb}")
                nc.gpsimd.dma_start(out=l0, in_=osb[64:65, :, :])
                linv1 = sbpool.tile([1, 2, 512], F32, tag="linv1",
                                    name=f"l1_{qb}")
                nc.vector.reciprocal_approx_fast(linv1, l0)
                linv = sbpool.tile([64, 2, 512], F32, tag="linv",
                                   name=f"li_{qb}")
                nc.gpsimd.partition_broadcast(linv, linv1)
                eps_prev = {"osb": osb, "linv": linv}
            else:
                eps_prev = {"osb": osb}

        # ---- tail for the last q-block: stay on-chip (no DMA bounces).
        # Broadcast l via a K=1 PE matmul from the base-64-aligned ones
        # column, reciprocal per head, then a head-split (K=64 accumulate)
        # projection that needs no cross-partition onorm assembly.
        osb = eps_prev["osb"]
        linv7 = []
        for h in range(2):
            lbp = smpool.tile([128, 512], F32, tag="sm", name=f"lbp_{h}")
            nc.tensor.matmul(lbp[0:64, :], ones65[64:65, :],
                             osb[64:65, h, :], start=True, stop=True)
            li = sbpool.tile([64, 512], F32, tag="linv7", name=f"li7_{h}")
            nc.vector.reciprocal_approx_fast(li, lbp[0:64, :])
            linv7.append(li)
        tmp7 = []
        for h in range(2):
            t7 = sbpool.tile([64, 512], BF16, tag="tmp7", name=f"t7_{h}")
            nc.vector.tensor_mul(t7, osb[0:64, h, :], linv7[h])
            tmp7.append(t7)
        ybuf7 = ybpool.tile([128, 4, C], BF16, tag="ybuf", name="yb_7")
        q0 = 512 * (NQB - 1)
        for s in range(4):
            for oc in range(2):
                yps = smpool.tile([128, 512], F32, tag="sm",
                                  name=f"yp7_{s}_{oc}")
                nc.tensor.matmul(yps, tmp7[0][:, 128 * s : 128 * (s + 1)],
                                 Wp[0:64, 512 * oc : 512 * (oc + 1)],
                                 start=True, stop=False)
                nc.tensor.matmul(yps, tmp7[1][:, 128 * s : 128 * (s + 1)],
                                 WpB[:, 512 * oc : 512 * (oc + 1)],
                                 start=False, stop=True)
                # alternate evac engines so the copy chain halves (scalar is
                # idle in the tail), and rotate DMA issues over 3 engines
                if oc == 0:
                    nc.vector.tensor_copy(
                        ybuf7[:, s, 512 * oc : 512 * (oc + 1)], yps)
                else:
                    nc.scalar.activation(
                        ybuf7[:, s, 512 * oc : 512 * (oc + 1)], yps, Copy)
                eng = (nc.sync, nc.scalar, nc.gpsimd)[(2 * s + oc) % 3]
                eng.dma_start(
                    out=y[q0 + 128 * s : q0 + 128 * s + 128,
                          512 * oc : 512 * (oc + 1)],
                    in_=ybuf7[:, s, 512 * oc : 512 * (oc + 1)])

        if taps is not None:
            nc.gpsimd.dma_start(out=taps["dQT"], in_=QT)
            nc.gpsimd.dma_start(out=taps["dKT"], in_=KT)
            nc.gpsimd.dma_start(
                out=taps["dVP"],
                in_=VP.rearrange("p t c -> p (t c)"))

    persist.release()


_NC_CACHE = {}


def _get_nc():
    if "nc" not in _NC_CACHE:
        _NC_CACHE["nc"] = build_nc()
    return _NC_CACHE["nc"]


def make_in_maps(x, w_qkv, w_proj):
    """Host-side sharding: per-core input dicts."""
    from concourse import mybir as _mb
    mdt = _mb.dt.np(BF16)
    xTh = x[0].T.astype(mdt)  # [C, N]
    # chunk-major, per-partition contiguous: xb[j, p, ct, n]
    xbh = np.ascontiguousarray(
        xTh.reshape(NCT, 128, NQB, 512).transpose(2, 1, 0, 3))
    tri = (np.arange(128)[None, :] >= np.arange(128)[:, None]).astype(mdt)
    in_maps = []
    for m in range(NCORES):
        r0 = HPC * D * m  # 128*m
        wq = w_qkv[r0 : r0 + 128]
        wk = w_qkv[C + r0 : C + r0 + 128]
        wvm = w_qkv[2 * C + r0 : 2 * C + r0 + 128]
        wqkT = np.concatenate([wq, wk], 0).T.astype(mdt)  # [C, 256]
        wvT = wvm.T.astype(mdt)                           # [C, 128]
        in_maps.append({
            "xb": xbh,
            "wqk": np.ascontiguousarray(
                wqkT.reshape(NCT, 128, 256).transpose(1, 0, 2)),
            "wv": np.ascontiguousarray(
                wvT.reshape(NCT, 128, 128).transpose(1, 0, 2)),
            "wp": np.ascontiguousarray(
                w_proj[:, r0 : r0 + 128].T.astype(mdt)),
            "tri": tri,
        })
    return in_maps


def kernel(x, w_qkv, w_proj, b_proj, _trace=False):
    x = np.asarray(x)
    w_qkv = np.asarray(w_qkv)
    w_proj = np.asarray(w_proj)
    b_proj = np.asarray(b_proj)
    nc = _get_nc()
    in_maps = make_in_maps(x, w_qkv, w_proj)
    res = run_bass_kernel_spmd(
        nc, in_maps, core_ids=list(range(NCORES)), trace=_trace
    )
    out = np.zeros((N, C), dtype=np.float32)
    for r in res.results:
        out += r["y"].astype(np.float32)
    out += b_proj.astype(np.float32)
    out = out.reshape(B, N, C)
    if _trace:
        return out, res
    return out



# revision 34
# speedup vs baseline: 1.0245x; 1.0245x over previous
"""Causal multi-head attention (B=1, N=4096, H=16, D=64) on 8 trn2 cores.

Head-parallel tensor parallelism: 2 heads per core.  Each core reads the
full x (pre-transposed on host), computes QKV for its 2 heads, runs causal
attention in the S^T (k-partition, q-free) layout, applies its 128-column
slice of the output projection, and writes a partial [4096, 1024] y that
the host sums (plus b_proj).

Single fused pipeline: the Q/K projection of chunk j+1, the V' projection
of chunk j, and the output projection of q-block j-1 are interleaved into
the attention kt-loop of q-block j as background PE work so the tensor
engine never idles (keeping its p-state at max).  Causal masking is
multiplicative on P after exp (off the S->exp critical path), diagonal
tiles are narrowed to their valid column range, V' is produced directly by
the V projection in [seq, d] layout (no transposes), 1/l is broadcast
across partitions via a small DMA + gpsimd partition_broadcast (mid
blocks) or a K=1 PE matmul + head-split projection (final block, keeping
the tail on-chip), x/weight loads are split across engines and queues with
chunk loads deferred until one block before use, and the y writes stream
out per 128x512 tile from the idle gpsimd engine.
"""

from collections import deque

import numpy as np

import concourse.bass as bass
from concourse import bacc, masks
import concourse.tile as tile
from concourse import mybir
from concourse.bass_utils import run_bass_kernel_spmd

B, N, H, D = 1, 4096, 16, 64
C = H * D  # 1024
SCALE = D ** -0.5
NCORES = 8
HPC = H // NCORES  # heads per core = 2
F32 = mybir.dt.float32
BF16 = mybir.dt.bfloat16

NKT = N // 128      # 32 k-tiles of 128
NQB = N // 512      # 8 q-blocks of 512
NCT = C // 128      # 8 contraction tiles for the projections


DEBUG_TAPS = False


def build_nc():
    nc = bacc.Bacc("TRN2", target_bir_lowering=False)

    # xb[j, p, ct, :]: chunk-major, per-partition contiguous (8KB descriptors)
    xb = nc.dram_tensor("xb", [NQB, 128, NCT, 512], BF16,
                        kind="ExternalInput").ap()
    wqk = nc.dram_tensor("wqk", [128, NCT, 256], BF16,
                         kind="ExternalInput").ap()
    wv = nc.dram_tensor("wv", [128, NCT, 128], BF16,
                        kind="ExternalInput").ap()
    wp = nc.dram_tensor("wp", [128, C], BF16, kind="ExternalInput").ap()
    tri = nc.dram_tensor("tri", [128, 128], BF16, kind="ExternalInput").ap()
    y = nc.dram_tensor("y", [N, C], BF16, kind="ExternalOutput").ap()

    taps = None
    if DEBUG_TAPS:
        taps = {
            "dL": nc.dram_tensor("dL", [NQB, 2, 512], F32,
                                 kind="ExternalOutput").ap(),
            "dLinv": nc.dram_tensor("dLinv", [NQB, 2, 512], F32,
                                    kind="ExternalOutput").ap(),
            "dOn": nc.dram_tensor("dOn", [NQB, 128, 512], BF16,
                                  kind="ExternalOutput").ap(),
            "dQT": nc.dram_tensor("dQT", [128, N], BF16,
                                  kind="ExternalOutput").ap(),
            "dKT": nc.dram_tensor("dKT", [128, N], BF16,
                                  kind="ExternalOutput").ap(),
            "dVP": nc.dram_tensor("dVP", [128, NKT * 130], BF16,
                                  kind="ExternalOutput").ap(),
        }

    with tile.TileContext(nc) as tc:
        _body(tc, xb, wqk, wv, wp, tri, y, taps)
    nc.compile()
    return nc


def _body(tc, xb, wqk, wv, wp, tri, y, taps=None):
    nc = tc.nc
    Exp = mybir.ActivationFunctionType.Exp
    Copy = mybir.ActivationFunctionType.Copy

    persist = tc.alloc_tile_pool(name="persist", bufs=1)

    # Persistent SBUF tensors
    xt = persist.tile([128, NCT, N], BF16, tag="xt")      # full x^T, resident
    QT = persist.tile([128, N], BF16, tag="QT")           # [(h,d), n]
    KT = persist.tile([128, N], BF16, tag="KT")
    # V' as one dedicated tile per (kt, head): the xbar transpose writes at
    # the destination TILE base (it drops any AP offset), so each transposed
    # d-block must be its own tile.  Col 64 holds the ones column (for l),
    # col 65 pads to 4B alignment.
    VPt = {(kt, h): persist.tile([128, 66], BF16, tag=f"vp_{kt}_{h}",
                                 name=f"vp_{kt}_{h}")
           for kt in range(NKT) for h in range(2)}
    Wqk = persist.tile([128, NCT, 256], BF16, tag="Wqk")
    Wv = persist.tile([128, NCT, 128], BF16, tag="Wv")
    Wp = persist.tile([128, C], BF16, tag="Wp")
    WpB = persist.tile([64, C], BF16, tag="WpB")          # Wp rows 64..127
    tri_sb = persist.tile([128, 128], BF16, tag="tri")    # [kp, qc] 1 if qc>=kp
    ones65 = persist.tile([65, 64], F32, tag="ones65")    # row 64 used as lhsT

    # startup: a single dma_start's descriptors land on ONE queue, so split
    # the critical chunk-0 / Wqk loads into several triggers spread across
    # engines (parallel queues).  Later chunks are issued an iteration ahead.
    def load_chunk(j, eng, nsplit=2):
        w = NCT // nsplit
        for i in range(nsplit):
            eng.dma_start(
                out=xt[:, w * i : w * (i + 1), 512 * j : 512 * (j + 1)],
                in_=xb[j, :, w * i : w * (i + 1), :])

    # the first trigger on each engine dispatches fastest: make them the two
    # pieces the very first matmul needs (x chunk-0 ct0 and Wqk ct0-1)
    nc.sync.dma_start(out=xt[:, 0:1, 0:512], in_=xb[0, :, 0:1, :])
    nc.scalar.dma_start(out=Wqk[:, 0:2, :], in_=wqk[:, 0:2, :])
    nc.sync.dma_start(out=Wqk[:, 2:5, :], in_=wqk[:, 2:5, :])
    nc.scalar.dma_start(out=xt[:, 1:2, 0:512], in_=xb[0, :, 1:2, :])
    nc.sync.dma_start(out=xt[:, 2:3, 0:512], in_=xb[0, :, 2:3, :])
    nc.scalar.dma_start(out=Wqk[:, 5:8, :], in_=wqk[:, 5:8, :])
    nc.scalar.dma_start(out=xt[:, 3:4, 0:512], in_=xb[0, :, 3:4, :])
    for i in range(4, 8):
        nc.gpsimd.dma_start(out=xt[:, i : i + 1, 0:512],
                            in_=xb[0, :, i : i + 1, :])
    nc.sync.dma_start(out=Wv[:, 0:4, :], in_=wv[:, 0:4, :])
    nc.scalar.dma_start(out=Wv[:, 4:8, :], in_=wv[:, 4:8, :])
    nc.sync.dma_start(out=tri_sb, in_=tri)
    load_chunk(1, nc.gpsimd, nsplit=2)
    nc.gpsimd.dma_start(out=Wp, in_=wp)
    nc.gpsimd.dma_start(out=WpB, in_=wp[64:128, :])
    nc.vector.memset(ones65, 1.0)

    # ones column of each V' tile (col 64; col 65 is don't-care padding)
    for kt in range(NKT):
        for h in range(2):
            nc.vector.memset(VPt[(kt, h)][:, 64:66], 1.0)

    with (
        tc.tile_pool(name="spool", bufs=2, space="PSUM") as spool,   # 4 banks
        tc.tile_pool(name="opool", bufs=1, space="PSUM") as opool,   # 2 banks
        tc.tile_pool(name="smpool", bufs=2, space="PSUM") as smpool, # 2 banks
        tc.tile_pool(name="ptpool", bufs=3) as ptpool,
        tc.tile_pool(name="sbpool", bufs=2) as sbpool,
        tc.tile_pool(name="ybpool", bufs=2) as ybpool,
        tc.tile_pool(name="vtpool", bufs=2) as vtpool,
    ):
        def qk_closure(j):
            """Q/K projection for sequence chunk j (out [och, seq]).  Q and K
            are interleaved so consecutive matmuls accumulate into different
            PSUM banks (avoids same-bank read-modify-write bubbles)."""
            cls = []
            holder = {}

            def qk_piece(c0):
                def run():
                    if c0 == 0:
                        holder["q"] = smpool.tile([128, 512], F32, tag="sm",
                                                  name=f"psq_{j}")
                        holder["k"] = smpool.tile([128, 512], F32, tag="sm",
                                                  name=f"psk_{j}")
                    psq, psk = holder["q"], holder["k"]
                    for ct in range(c0, c0 + 2):
                        nc.tensor.matmul(
                            psq, Wqk[:, ct, 0:128],
                            xt[:, ct, 512 * j : 512 * (j + 1)],
                            start=(ct == 0), stop=(ct == NCT - 1),
                        )
                        nc.tensor.matmul(
                            psk, Wqk[:, ct, 128:256],
                            xt[:, ct, 512 * j : 512 * (j + 1)],
                            start=(ct == 0), stop=(ct == NCT - 1),
                        )
                    if c0 == NCT - 2:
                        nc.vector.tensor_copy(QT[:, 512 * j : 512 * (j + 1)],
                                              psq)
                        nc.vector.tensor_copy(KT[:, 512 * j : 512 * (j + 1)],
                                              psk)
                return run

            for c0 in range(0, NCT, 2):
                cls.append(qk_piece(c0))
            return cls

        def v_closures(j):
            """V for chunk j, computed as V^T [(h d), seq] with N=512
            matmuls (4x fewer PE issues than the [seq, d] form), then
            rotated into the V' [seq, d] layout by the DMA xbar transpose
            (sync engine; dst offset must be 0, hence the staging tile)
            plus a cheap 4x-mode DVE copy."""
            cls = []
            holder = {}

            def vt_piece(c0):
                def run():
                    if c0 == 0:
                        holder["vt"] = smpool.tile([128, 512], F32, tag="sm",
                                                   name=f"vtps_{j}")
                    ps = holder["vt"]
                    for ct in range(c0, c0 + 2):
                        nc.tensor.matmul(
                            ps, Wv[:, ct, :],
                            xt[:, ct, 512 * j : 512 * (j + 1)],
                            start=(ct == 0), stop=(ct == NCT - 1),
                        )
                    if c0 == NCT - 2:
                        vts = vtpool.tile([128, 512], BF16, tag="vts",
                                          name=f"vts_{j}")
                        nc.vector.tensor_copy(vts, ps)
                        for i in range(4):
                            kt = 4 * j + i
                            for h in range(2):
                                nc.sync.dma_start_transpose(
                                    VPt[(kt, h)][:, 0:64],
                                    vts[64 * h : 64 * h + 64,
                                        128 * i : 128 * (i + 1)])
                return run

            for c0 in range(0, NCT, 2):
                cls.append(vt_piece(c0))
            return cls

        def proj_closures(qb, eps):
            """Normalize o by 1/l (PE partition-broadcast) and project."""
            cls = []
            onorm = sbpool.tile([128, 512], BF16, tag="onorm",
                                name=f"onorm_{qb}")

            def norm_mul(h):
                def run():
                    if h == 0:
                        nc.vector.tensor_mul(onorm[0:64, :],
                                             eps["osb"][0:64, 0, :],
                                             eps["linv"][:, 0, :])
                    else:
                        tmp = sbpool.tile([64, 512], BF16, tag="tmp",
                                          name=f"tmp_{qb}")
                        nc.vector.tensor_mul(tmp, eps["osb"][0:64, 1, :],
                                             eps["linv"][:, 1, :])
                        nc.sync.dma_start(out=onorm[64:128, :], in_=tmp)
                        if taps is not None:
                            nc.gpsimd.dma_start(out=taps["dL"][qb],
                                                in_=eps["osb"][64:65, :, :])
                            nc.gpsimd.dma_start(out=taps["dLinv"][qb],
                                                in_=eps["linv"][0:1, :, :])
                            nc.gpsimd.dma_start(out=taps["dOn"][qb], in_=onorm)
                return run

            cls.append(norm_mul(0))
            cls.append(norm_mul(1))

            ybuf = ybpool.tile([128, 4, C], BF16, tag="ybuf", name=f"yb_{qb}")

            def proj(s, oc):
                def run():
                    yps = smpool.tile([128, 512], F32, tag="sm")
                    nc.tensor.matmul(
                        yps, onorm[:, 128 * s : 128 * (s + 1)],
                        Wp[:, 512 * oc : 512 * (oc + 1)],
                        start=True, stop=True,
                    )
                    nc.vector.tensor_copy(ybuf[:, s, 512 * oc : 512 * (oc + 1)],
                                          yps)
                    q0 = 512 * qb + 128 * s
                    nc.gpsimd.dma_start(
                        out=y[q0 : q0 + 128, 512 * oc : 512 * (oc + 1)],
                        in_=ybuf[:, s, 512 * oc : 512 * (oc + 1)])
                return run

            for s in range(4):
                for oc in range(2):
                    cls.append(proj(s, oc))
            return cls

        # ---- fused main loop ----
        bg = deque(qk_closure(0) + v_closures(0) + v_closures(1))
        while bg:
            bg.popleft()()

        eps_prev = None
        for qb in range(NQB):
            n_kt = 4 * (qb + 1)
            q0 = 512 * qb

            if qb + 2 < NQB:
                load_chunk(qb + 2, nc.gpsimd)

            # background PE work for this block's kt loop: Q/K of the next
            # chunk (needed right at the next block's start), V' TWO chunks
            # ahead (its matmul -> evac -> xbar-transpose chain is long, so
            # give it a full block of slack), and the previous block's
            # projection.
            bg = deque()
            if qb + 1 < NQB:
                bg.extend(qk_closure(qb + 1))
            if qb + 2 < NQB:
                bg.extend(v_closures(qb + 2))
            if eps_prev is not None:
                bg.extend(proj_closures(qb - 1, eps_prev))

            o_ps = opool.tile([65, 2, 512], F32, tag="o", name=f"o_{qb}")
            s_tiles = {}

            def emit_S(kt, qb=qb, q0=q0, s_tiles=s_tiles):
                jr = kt - 4 * qb
                qs = 128 * jr if jr >= 0 else 0
                s_ps = spool.tile([128, 2, 512], F32, tag="s",
                                  name=f"s_{qb}_{kt}")
                for h in range(2):
                    nc.tensor.matmul(
                        s_ps[:, h, qs:512],
                        KT[64 * h : 64 * h + 64, 128 * kt : 128 * (kt + 1)],
                        QT[64 * h : 64 * h + 64, q0 + qs : q0 + 512],
                        start=True, stop=True,
                    )
                s_tiles[kt] = (s_ps, qs)

            emit_S(0)
            bg0 = len(bg)
            drained = 0
            for kt in range(n_kt):
                if kt + 1 < n_kt:
                    emit_S(kt + 1)
                s_ps, qs = s_tiles.pop(kt)
                pt = ptpool.tile([128, 2, 512], BF16, tag="pt",
                                 name=f"pt_{qb}_{kt}")
                nc.scalar.activation(pt[:, :, qs:512], s_ps[:, :, qs:512],
                                     Exp, scale=SCALE)
                if kt - 4 * qb >= 0:  # diagonal tile: zero invalid triangle
                    # one op for both heads; tri broadcast over the h dim
                    # via a stride-0 middle AP dim
                    tri_b = bass.AP(tensor=tri_sb.tensor, offset=tri_sb.offset,
                                    ap=[list(tri_sb.ap[0]), [0, 2],
                                        list(tri_sb.ap[1])])
                    nc.vector.tensor_mul(pt[:, :, qs : qs + 128],
                                         pt[:, :, qs : qs + 128], tri_b)
                for h in range(2):
                    nc.tensor.matmul(
                        o_ps[:, h, qs:512], VPt[(kt, h)][:, 0:65],
                        pt[:, h, qs:512],
                        start=(kt == 0), stop=(kt == n_kt - 1),
                    )
                # drain background PE work spread uniformly over ALL
                # iterations (ceil-of-remaining exhausts the queue early and
                # starves the tail iterations of filler work)
                target = -(-(kt + 1) * bg0 // n_kt)
                while drained < target and bg:
                    bg.popleft()()
                    drained += 1

            # epilogue: free o_ps via vector copy (scalar is the exp
            # bottleneck engine; gpsimd cannot read PSUM).  For the last
            # block, split so the l row (scalar) lands in parallel with the
            # d rows (vector) -- shortens the tail's serial chain.
            osb = sbpool.tile([65, 2, 512], F32, tag="osb", name=f"osb_{qb}")
            if qb == NQB - 1:
                nc.scalar.activation(osb[64:65, :, :], o_ps[64:65, :, :],
                                     Copy)
                nc.vector.tensor_copy(osb[0:64, :, :], o_ps[0:64, :, :])
            else:
                nc.vector.tensor_copy(osb, o_ps)
            if qb < NQB - 1:
                # move the l row to partition 0 (small DMA), 1/l there, then
                # gpsimd partition_broadcast across partitions 0..63 (DVE
                # lanes cannot cross partitions / need quad-aligned bases)
                # gpsimd, not sync: the sync queue carries the xbar
                # transposes, which must not delay this latency-critical hop
                l0 = sbpool.tile([1, 2, 512], F32, tag="l0", name=f"l0_{qb}")
                nc.gpsimd.dma_start(out=l0, in_=osb[64:65, :, :])
                linv1 = sbpool.tile([1, 2, 512], F32, tag="linv1",
                                    name=f"l1_{qb}")
                nc.vector.reciprocal_approx_fast(linv1, l0)
                linv = sbpool.tile([64, 2, 512], F32, tag="linv",
                                   name=f"li_{qb}")
                nc.gpsimd.partition_broadcast(linv, linv1)
                eps_prev = {"osb": osb, "linv": linv}
            else:
                eps_prev = {"osb": osb}

        # ---- tail for the last q-block: stay on-chip (no DMA bounces).
        # Broadcast l via a K=1 PE matmul from the base-64-aligned ones
        # column, reciprocal per head, then a head-split (K=64 accumulate)
        # projection that needs no cross-partition onorm assembly.
        osb = eps_prev["osb"]
        linv7 = []
        for h in range(2):
            lbp = smpool.tile([128, 512], F32, tag="sm", name=f"lbp_{h}")
            nc.tensor.matmul(lbp[0:64, :], ones65[64:65, :],
                             osb[64:65, h, :], start=True, stop=True)
            li = sbpool.tile([64, 512], F32, tag="linv7", name=f"li7_{h}")
            nc.vector.reciprocal_approx_fast(li, lbp[0:64, :])
            linv7.append(li)
        tmp7 = []
        for h in range(2):
            t7 = sbpool.tile([64, 512], BF16, tag="tmp7", name=f"t7_{h}")
            nc.vector.tensor_mul(t7, osb[0:64, h, :], linv7[h])
            tmp7.append(t7)
        ybuf7 = ybpool.tile([128, 4, C], BF16, tag="ybuf", name="yb_7")
        q0 = 512 * (NQB - 1)
        for s in range(4):
            for oc in range(2):
                yps = smpool.tile([128, 512], F32, tag="sm",
                                  name=f"yp7_{s}_{oc}")
                nc.tensor.matmul(yps, tmp7[0][:, 128 * s : 128 * (s + 1)],
                                 Wp[0:64, 512 * oc : 512 * (oc + 1)],
                                 start=True, stop=False)
                nc.tensor.matmul(yps, tmp7[1][:, 128 * s : 128 * (s + 1)],
                                 WpB[:, 512 * oc : 512 * (oc + 1)],
                                 start=False, stop=True)
                # alternate evac engines so the copy chain halves (scalar is
                # idle in the tail), and rotate DMA issues over 3 engines
                if oc == 0:
                    nc.vector.tensor_copy(
                        ybuf7[:, s, 512 * oc : 512 * (oc + 1)], yps)
                else:
                    nc.scalar.activation(
                        ybuf7[:, s, 512 * oc : 512 * (oc + 1)], yps, Copy)
                eng = (nc.sync, nc.scalar, nc.gpsimd)[(2 * s + oc) % 3]
                eng.dma_start(
                    out=y[q0 + 128 * s : q0 + 128 * s + 128,
                          512 * oc : 512 * (oc + 1)],
                    in_=ybuf7[:, s, 512 * oc : 512 * (oc + 1)])

        if taps is not None:
            nc.gpsimd.dma_start(out=taps["dQT"], in_=QT)
            nc.gpsimd.dma_start(out=taps["dKT"], in_=KT)
            pass  # dVP tap removed (V' is now per-(kt,h) tiles)

    persist.release()


_NC_CACHE = {}


def _get_nc():
    if "nc" not in _NC_CACHE:
        _NC_CACHE["nc"] = build_nc()
    return _NC_CACHE["nc"]


def make_in_maps(x, w_qkv, w_proj):
    """Host-side sharding: per-core input dicts."""
    from concourse import mybir as _mb
    mdt = _mb.dt.np(BF16)
    xTh = x[0].T.astype(mdt)  # [C, N]
    # chunk-major, per-partition contiguous: xb[j, p, ct, n]
    xbh = np.ascontiguousarray(
        xTh.reshape(NCT, 128, NQB, 512).transpose(2, 1, 0, 3))
    tri = (np.arange(128)[None, :] >= np.arange(128)[:, None]).astype(mdt)
    in_maps = []
    for m in range(NCORES):
        r0 = HPC * D * m  # 128*m
        wq = w_qkv[r0 : r0 + 128]
        wk = w_qkv[C + r0 : C + r0 + 128]
        wvm = w_qkv[2 * C + r0 : 2 * C + r0 + 128]
        wqkT = np.concatenate([wq, wk], 0).T.astype(mdt)  # [C, 256]
        wvT = wvm.T.astype(mdt)                           # [C, 128]
        in_maps.append({
            "xb": xbh,
            "wqk": np.ascontiguousarray(
                wqkT.reshape(NCT, 128, 256).transpose(1, 0, 2)),
            "wv": np.ascontiguousarray(
                wvT.reshape(NCT, 128, 128).transpose(1, 0, 2)),
            "wp": np.ascontiguousarray(
                w_proj[:, r0 : r0 + 128].T.astype(mdt)),
            "tri": tri,
        })
    return in_maps


def kernel(x, w_qkv, w_proj, b_proj, _trace=False):
    x = np.asarray(x)
    w_qkv = np.asarray(w_qkv)
    w_proj = np.asarray(w_proj)
    b_proj = np.asarray(b_proj)
    nc = _get_nc()
    in_maps = make_in_maps(x, w_qkv, w_proj)
    res = run_bass_kernel_spmd(
        nc, in_maps, core_ids=list(range(NCORES)), trace=_trace
    )
    out = np.zeros((N, C), dtype=np.float32)
    for r in res.results:
        out += r["y"].astype(np.float32)
    out += b_proj.astype(np.float32)
    out = out.reshape(B, N, C)
    if _trace:
        return out, res
    return out



# revision 36
# speedup vs baseline: 1.3247x; 1.2930x over previous
"""Causal multi-head attention (B=1, N=4096, H=16, D=64) on 8 trn2 cores.

Head-parallel tensor parallelism: 2 heads per core.  Each core reads the
full x (pre-transposed on host), computes QKV for its 2 heads, runs causal
attention in the S^T (k-partition, q-free) layout, applies its 128-column
slice of the output projection, and writes a partial [4096, 1024] y that
the host sums (plus b_proj).

Single fused pipeline: the Q/K projection of chunk j+1, the V' projection
of chunk j, and the output projection of q-block j-1 are interleaved into
the attention kt-loop of q-block j as background PE work so the tensor
engine never idles (keeping its p-state at max).  Causal masking is
multiplicative on P after exp (off the S->exp critical path), diagonal
tiles are narrowed to their valid column range, V' is produced directly by
the V projection in [seq, d] layout (no transposes), 1/l is broadcast
across partitions via a small DMA + gpsimd partition_broadcast (mid
blocks) or a K=1 PE matmul + head-split projection (final block, keeping
the tail on-chip), x/weight loads are split across engines and queues with
chunk loads deferred until one block before use, and the y writes stream
out per 128x512 tile from the idle gpsimd engine.
"""

from collections import deque

import numpy as np

import concourse.bass as bass
from concourse import bacc, masks
import concourse.tile as tile
from concourse import mybir
from concourse.bass_utils import run_bass_kernel_spmd

B, N, H, D = 1, 4096, 16, 64
C = H * D  # 1024
SCALE = D ** -0.5
NCORES = 8
HPC = H // NCORES  # heads per core = 2
F32 = mybir.dt.float32
BF16 = mybir.dt.bfloat16

NKT = N // 128      # 32 k-tiles of 128
NQB = N // 512      # 8 q-blocks of 512
NCT = C // 128      # 8 contraction tiles for the projections


DEBUG_TAPS = False


def build_nc():
    nc = bacc.Bacc("TRN2", target_bir_lowering=False)

    # xb[j, p, ct, :]: chunk-major, per-partition contiguous (8KB descriptors)
    xb = nc.dram_tensor("xb", [NQB, 128, NCT, 512], BF16,
                        kind="ExternalInput").ap()
    wqk = nc.dram_tensor("wqk", [128, NCT, 256], BF16,
                         kind="ExternalInput").ap()
    wv = nc.dram_tensor("wv", [128, NCT, 128], BF16,
                        kind="ExternalInput").ap()
    wp = nc.dram_tensor("wp", [128, C], BF16, kind="ExternalInput").ap()
    tri = nc.dram_tensor("tri", [128, 128], BF16, kind="ExternalInput").ap()
    y = nc.dram_tensor("y", [N, C], BF16, kind="ExternalOutput").ap()

    taps = None
    if DEBUG_TAPS:
        taps = {
            "dL": nc.dram_tensor("dL", [NQB, 2, 512], F32,
                                 kind="ExternalOutput").ap(),
            "dLinv": nc.dram_tensor("dLinv", [NQB, 2, 512], F32,
                                    kind="ExternalOutput").ap(),
            "dOn": nc.dram_tensor("dOn", [NQB, 128, 512], BF16,
                                  kind="ExternalOutput").ap(),
            "dQT": nc.dram_tensor("dQT", [128, N], BF16,
                                  kind="ExternalOutput").ap(),
            "dKT": nc.dram_tensor("dKT", [128, N], BF16,
                                  kind="ExternalOutput").ap(),
            "dVP": nc.dram_tensor("dVP", [128, NKT * 130], BF16,
                                  kind="ExternalOutput").ap(),
        }

    with tile.TileContext(nc) as tc:
        _body(tc, xb, wqk, wv, wp, tri, y, taps)
    nc.compile()
    return nc


def _body(tc, xb, wqk, wv, wp, tri, y, taps=None):
    nc = tc.nc
    Exp = mybir.ActivationFunctionType.Exp
    Copy = mybir.ActivationFunctionType.Copy

    persist = tc.alloc_tile_pool(name="persist", bufs=1)

    # Persistent SBUF tensors
    xt = persist.tile([128, NCT, N], BF16, tag="xt")      # full x^T, resident
    QT = persist.tile([128, N], BF16, tag="QT")           # [(h,d), n]
    KT = persist.tile([128, N], BF16, tag="KT")
    # V' as one dedicated tile per (kt, head): the xbar transpose writes at
    # the destination TILE base (it drops any AP offset), so each transposed
    # d-block must be its own tile.  Col 64 holds the ones column (for l),
    # col 65 pads to 4B alignment.
    VPt = {(kt, h): persist.tile([128, 66], BF16, tag=f"vp_{kt}_{h}",
                                 name=f"vp_{kt}_{h}")
           for kt in range(NKT) for h in range(2)}
    Wqk = persist.tile([128, NCT, 256], BF16, tag="Wqk")
    Wv = persist.tile([128, NCT, 128], BF16, tag="Wv")
    Wp = persist.tile([128, C], BF16, tag="Wp")
    WpB = persist.tile([64, C], BF16, tag="WpB")          # Wp rows 64..127
    tri_sb = persist.tile([128, 128], BF16, tag="tri")    # [kp, qc] 1 if qc>=kp
    ones65 = persist.tile([65, 64], F32, tag="ones65")    # row 64 used as lhsT

    # startup: a single dma_start's descriptors land on ONE queue, so split
    # the critical chunk-0 / Wqk loads into several triggers spread across
    # engines (parallel queues).  Later chunks are issued an iteration ahead.
    def load_chunk(j, eng, nsplit=2):
        w = NCT // nsplit
        for i in range(nsplit):
            eng.dma_start(
                out=xt[:, w * i : w * (i + 1), 512 * j : 512 * (j + 1)],
                in_=xb[j, :, w * i : w * (i + 1), :])

    # the first trigger on each engine dispatches fastest: make them the two
    # pieces the very first matmul needs (x chunk-0 ct0 and Wqk ct0-1)
    nc.sync.dma_start(out=xt[:, 0:1, 0:512], in_=xb[0, :, 0:1, :])
    nc.scalar.dma_start(out=Wqk[:, 0:2, :], in_=wqk[:, 0:2, :])
    nc.sync.dma_start(out=Wqk[:, 2:5, :], in_=wqk[:, 2:5, :])
    nc.scalar.dma_start(out=xt[:, 1:2, 0:512], in_=xb[0, :, 1:2, :])
    nc.sync.dma_start(out=xt[:, 2:3, 0:512], in_=xb[0, :, 2:3, :])
    nc.scalar.dma_start(out=Wqk[:, 5:8, :], in_=wqk[:, 5:8, :])
    nc.scalar.dma_start(out=xt[:, 3:4, 0:512], in_=xb[0, :, 3:4, :])
    for i in range(4, 8):
        nc.gpsimd.dma_start(out=xt[:, i : i + 1, 0:512],
                            in_=xb[0, :, i : i + 1, :])
    nc.sync.dma_start(out=Wv[:, 0:4, :], in_=wv[:, 0:4, :])
    nc.scalar.dma_start(out=Wv[:, 4:8, :], in_=wv[:, 4:8, :])
    nc.sync.dma_start(out=tri_sb, in_=tri)
    load_chunk(1, nc.gpsimd, nsplit=2)
    nc.gpsimd.dma_start(out=Wp, in_=wp)
    nc.gpsimd.dma_start(out=WpB, in_=wp[64:128, :])
    nc.vector.memset(ones65, 1.0)

    # ones column of each V' tile (col 64; col 65 is don't-care padding)
    for kt in range(NKT):
        for h in range(2):
            nc.vector.memset(VPt[(kt, h)][:, 64:66], 1.0)

    with (
        tc.tile_pool(name="spool", bufs=2, space="PSUM") as spool,   # 4 banks
        tc.tile_pool(name="opool", bufs=1, space="PSUM") as opool,   # 2 banks
        tc.tile_pool(name="smpool", bufs=2, space="PSUM") as smpool, # 2 banks
        tc.tile_pool(name="ptpool", bufs=3) as ptpool,
        tc.tile_pool(name="sbpool", bufs=2) as sbpool,
        tc.tile_pool(name="ybpool", bufs=2) as ybpool,
    ):
        def qk_closure(j):
            """Q/K projection for sequence chunk j (out [och, seq]).  Q and K
            are interleaved so consecutive matmuls accumulate into different
            PSUM banks (avoids same-bank read-modify-write bubbles)."""
            cls = []
            holder = {}

            def qk_piece(c0):
                def run():
                    if c0 == 0:
                        holder["q"] = smpool.tile([128, 512], F32, tag="sm",
                                                  name=f"psq_{j}")
                        holder["k"] = smpool.tile([128, 512], F32, tag="sm",
                                                  name=f"psk_{j}")
                    psq, psk = holder["q"], holder["k"]
                    for ct in range(c0, c0 + 2):
                        nc.tensor.matmul(
                            psq, Wqk[:, ct, 0:128],
                            xt[:, ct, 512 * j : 512 * (j + 1)],
                            start=(ct == 0), stop=(ct == NCT - 1),
                        )
                        nc.tensor.matmul(
                            psk, Wqk[:, ct, 128:256],
                            xt[:, ct, 512 * j : 512 * (j + 1)],
                            start=(ct == 0), stop=(ct == NCT - 1),
                        )
                    if c0 == NCT - 2:
                        nc.vector.tensor_copy(QT[:, 512 * j : 512 * (j + 1)],
                                              psq)
                        nc.vector.tensor_copy(KT[:, 512 * j : 512 * (j + 1)],
                                              psk)
                return run

            for c0 in range(0, NCT, 2):
                cls.append(qk_piece(c0))
            return cls

        def v_closures(j):
            """V' for chunk j directly in [seq, d] layout via N=128 matmuls
            (PE); pairs accumulate into different PSUM banks."""
            cls = []
            holder = {}

            def vchunk(s0, c0):
                def run():
                    if c0 == 0:
                        holder[f"v{s0}"] = [
                            smpool.tile([128, 512], F32, tag="sm",
                                        name=f"vps_{j}_{s0}_{i}")
                            for i in range(2)]
                    ps = holder[f"v{s0}"]
                    for ct in range(c0, c0 + 4):
                        for i in range(2):
                            kt = 4 * j + s0 + i
                            nc.tensor.matmul(
                                ps[i][:, 0:128],
                                xt[:, ct, 128 * kt : 128 * (kt + 1)],
                                Wv[:, ct, :],
                                start=(ct == 0), stop=(ct == NCT - 1),
                            )
                    if c0 == NCT - 4:
                        for i in range(2):
                            kt = 4 * j + s0 + i
                            for h in range(2):
                                nc.vector.tensor_copy(
                                    VPt[(kt, h)][:, 0:64],
                                    ps[i][:, 64 * h : 64 * h + 64])
                return run

            for s0 in (0, 2):
                for c0 in (0, 4):
                    cls.append(vchunk(s0, c0))
            return cls

        def proj_closures(qb, eps):
            """Normalize o by 1/l (PE partition-broadcast) and project."""
            cls = []
            onorm = sbpool.tile([128, 512], BF16, tag="onorm",
                                name=f"onorm_{qb}")

            def norm_mul(h):
                def run():
                    if h == 0:
                        nc.vector.tensor_mul(onorm[0:64, :],
                                             eps["osb"][0:64, 0, :],
                                             eps["linv"][:, 0, :])
                    else:
                        tmp = sbpool.tile([64, 512], BF16, tag="tmp",
                                          name=f"tmp_{qb}")
                        nc.vector.tensor_mul(tmp, eps["osb"][0:64, 1, :],
                                             eps["linv"][:, 1, :])
                        nc.sync.dma_start(out=onorm[64:128, :], in_=tmp)
                        if taps is not None:
                            nc.gpsimd.dma_start(out=taps["dL"][qb],
                                                in_=eps["osb"][64:65, :, :])
                            nc.gpsimd.dma_start(out=taps["dLinv"][qb],
                                                in_=eps["linv"][0:1, :, :])
                            nc.gpsimd.dma_start(out=taps["dOn"][qb], in_=onorm)
                return run

            cls.append(norm_mul(0))
            cls.append(norm_mul(1))

            ybuf = ybpool.tile([128, 4, C], BF16, tag="ybuf", name=f"yb_{qb}")

            def proj(s, oc):
                def run():
                    yps = smpool.tile([128, 512], F32, tag="sm")
                    nc.tensor.matmul(
                        yps, onorm[:, 128 * s : 128 * (s + 1)],
                        Wp[:, 512 * oc : 512 * (oc + 1)],
                        start=True, stop=True,
                    )
                    nc.vector.tensor_copy(ybuf[:, s, 512 * oc : 512 * (oc + 1)],
                                          yps)
                    q0 = 512 * qb + 128 * s
                    nc.gpsimd.dma_start(
                        out=y[q0 : q0 + 128, 512 * oc : 512 * (oc + 1)],
                        in_=ybuf[:, s, 512 * oc : 512 * (oc + 1)])
                return run

            for s in range(4):
                for oc in range(2):
                    cls.append(proj(s, oc))
            return cls

        # ---- fused main loop ----
        bg = deque(qk_closure(0) + v_closures(0))
        while bg:
            bg.popleft()()

        eps_prev = None
        for qb in range(NQB):
            n_kt = 4 * (qb + 1)
            q0 = 512 * qb

            if qb + 2 < NQB:
                load_chunk(qb + 2, nc.gpsimd)

            # background PE work for this block's kt loop: Q/K of the next
            # chunk (needed right at the next block's start), V' TWO chunks
            # ahead (its matmul -> evac -> xbar-transpose chain is long, so
            # give it a full block of slack), and the previous block's
            # projection.
            bg = deque()
            if qb + 1 < NQB:
                bg.extend(qk_closure(qb + 1))
                bg.extend(v_closures(qb + 1))
            if eps_prev is not None:
                bg.extend(proj_closures(qb - 1, eps_prev))

            o_ps = opool.tile([65, 2, 512], F32, tag="o", name=f"o_{qb}")
            s_tiles = {}

            def emit_S(kt, qb=qb, q0=q0, s_tiles=s_tiles):
                jr = kt - 4 * qb
                qs = 128 * jr if jr >= 0 else 0
                s_ps = spool.tile([128, 2, 512], F32, tag="s",
                                  name=f"s_{qb}_{kt}")
                for h in range(2):
                    nc.tensor.matmul(
                        s_ps[:, h, qs:512],
                        KT[64 * h : 64 * h + 64, 128 * kt : 128 * (kt + 1)],
                        QT[64 * h : 64 * h + 64, q0 + qs : q0 + 512],
                        start=True, stop=True,
                    )
                s_tiles[kt] = (s_ps, qs)

            emit_S(0)
            bg0 = len(bg)
            drained = 0
            for kt in range(n_kt):
                if kt + 1 < n_kt:
                    emit_S(kt + 1)
                s_ps, qs = s_tiles.pop(kt)
                pt = ptpool.tile([128, 2, 512], BF16, tag="pt",
                                 name=f"pt_{qb}_{kt}")
                nc.scalar.activation(pt[:, :, qs:512], s_ps[:, :, qs:512],
                                     Exp, scale=SCALE)
                if kt - 4 * qb >= 0:  # diagonal tile: zero invalid triangle
                    # one op for both heads; tri broadcast over the h dim
                    # via a stride-0 middle AP dim
                    tri_b = bass.AP(tensor=tri_sb.tensor, offset=tri_sb.offset,
                                    ap=[list(tri_sb.ap[0]), [0, 2],
                                        list(tri_sb.ap[1])])
                    nc.vector.tensor_mul(pt[:, :, qs : qs + 128],
                                         pt[:, :, qs : qs + 128], tri_b)
                for h in range(2):
                    nc.tensor.matmul(
                        o_ps[:, h, qs:512], VPt[(kt, h)][:, 0:65],
                        pt[:, h, qs:512],
                        start=(kt == 0), stop=(kt == n_kt - 1),
                    )
                # drain background PE work spread uniformly over ALL
                # iterations (ceil-of-remaining exhausts the queue early and
                # starves the tail iterations of filler work)
                target = -(-(kt + 1) * bg0 // n_kt)
                while drained < target and bg:
                    bg.popleft()()
                    drained += 1

            # epilogue: free o_ps via vector copy (scalar is the exp
            # bottleneck engine; gpsimd cannot read PSUM).  For the last
            # block, split so the l row (scalar) lands in parallel with the
            # d rows (vector) -- shortens the tail's serial chain.
            osb = sbpool.tile([65, 2, 512], F32, tag="osb", name=f"osb_{qb}")
            if qb == NQB - 1:
                nc.scalar.activation(osb[64:65, :, :], o_ps[64:65, :, :],
                                     Copy)
                nc.vector.tensor_copy(osb[0:64, :, :], o_ps[0:64, :, :])
            else:
                nc.vector.tensor_copy(osb, o_ps)
            if qb < NQB - 1:
                # move the l row to partition 0 (small DMA), 1/l there, then
                # gpsimd partition_broadcast across partitions 0..63 (DVE
                # lanes cannot cross partitions / need quad-aligned bases)
                # gpsimd, not sync: the sync queue carries the xbar
                # transposes, which must not delay this latency-critical hop
                l0 = sbpool.tile([1, 2, 512], F32, tag="l0", name=f"l0_{qb}")
                nc.gpsimd.dma_start(out=l0, in_=osb[64:65, :, :])
                linv1 = sbpool.tile([1, 2, 512], F32, tag="linv1",
                                    name=f"l1_{qb}")
                nc.vector.reciprocal_approx_fast(linv1, l0)
                linv = sbpool.tile([64, 2, 512], F32, tag="linv",
                                   name=f"li_{qb}")
                nc.gpsimd.partition_broadcast(linv, linv1)
                eps_prev = {"osb": osb, "linv": linv}
            else:
                eps_prev = {"osb": osb}

        # ---- tail for the last q-block: stay on-chip (no DMA bounces).
        # Broadcast l via a K=1 PE matmul from the base-64-aligned ones
        # column, reciprocal per head, then a head-split (K=64 accumulate)
        # projection that needs no cross-partition onorm assembly.
        osb = eps_prev["osb"]
        linv7 = []
        for h in range(2):
            lbp = smpool.tile([128, 512], F32, tag="sm", name=f"lbp_{h}")
            nc.tensor.matmul(lbp[0:64, :], ones65[64:65, :],
                             osb[64:65, h, :], start=True, stop=True)
            li = sbpool.tile([64, 512], F32, tag="linv7", name=f"li7_{h}")
            nc.vector.reciprocal_approx_fast(li, lbp[0:64, :])
            linv7.append(li)
        tmp7 = []
        for h in range(2):
            t7 = sbpool.tile([64, 512], BF16, tag="tmp7", name=f"t7_{h}")
            nc.vector.tensor_mul(t7, osb[0:64, h, :], linv7[h])
            tmp7.append(t7)
        ybuf7 = ybpool.tile([128, 4, C], BF16, tag="ybuf", name="yb_7")
        q0 = 512 * (NQB - 1)
        for s in range(4):
            for oc in range(2):
                yps = smpool.tile([128, 512], F32, tag="sm",
                                  name=f"yp7_{s}_{oc}")
                nc.tensor.matmul(yps, tmp7[0][:, 128 * s : 128 * (s + 1)],
                                 Wp[0:64, 512 * oc : 512 * (oc + 1)],
                                 start=True, stop=False)
                nc.tensor.matmul(yps, tmp7[1][:, 128 * s : 128 * (s + 1)],
                                 WpB[:, 512 * oc : 512 * (oc + 1)],
                                 start=False, stop=True)
                # alternate evac engines so the copy chain halves (scalar is
                # idle in the tail), and rotate DMA issues over 3 engines
                if oc == 0:
                    nc.vector.tensor_copy(
                        ybuf7[:, s, 512 * oc : 512 * (oc + 1)], yps)
                else:
                    nc.scalar.activation(
                        ybuf7[:, s, 512 * oc : 512 * (oc + 1)], yps, Copy)
                eng = (nc.sync, nc.scalar, nc.gpsimd)[(2 * s + oc) % 3]
                eng.dma_start(
                    out=y[q0 + 128 * s : q0 + 128 * s + 128,
                          512 * oc : 512 * (oc + 1)],
                    in_=ybuf7[:, s, 512 * oc : 512 * (oc + 1)])

        if taps is not None:
            nc.gpsimd.dma_start(out=taps["dQT"], in_=QT)
            nc.gpsimd.dma_start(out=taps["dKT"], in_=KT)
            pass  # dVP tap removed (V' is now per-(kt,h) tiles)

    persist.release()


_NC_CACHE = {}


def _get_nc():
    if "nc" not in _NC_CACHE:
        _NC_CACHE["nc"] = build_nc()
    return _NC_CACHE["nc"]


def make_in_maps(x, w_qkv, w_proj):
    """Host-side sharding: per-core input dicts."""
    from concourse import mybir as _mb
    mdt = _mb.dt.np(BF16)
    xTh = x[0].T.astype(mdt)  # [C, N]
    # chunk-major, per-partition contiguous: xb[j, p, ct, n]
    xbh = np.ascontiguousarray(
        xTh.reshape(NCT, 128, NQB, 512).transpose(2, 1, 0, 3))
    tri = (np.arange(128)[None, :] >= np.arange(128)[:, None]).astype(mdt)
    in_maps = []
    for m in range(NCORES):
        r0 = HPC * D * m  # 128*m
        wq = w_qkv[r0 : r0 + 128]
        wk = w_qkv[C + r0 : C + r0 + 128]
        wvm = w_qkv[2 * C + r0 : 2 * C + r0 + 128]
        wqkT = np.concatenate([wq, wk], 0).T.astype(mdt)  # [C, 256]
        wvT = wvm.T.astype(mdt)                           # [C, 128]
        in_maps.append({
            "xb": xbh,
            "wqk": np.ascontiguousarray(
                wqkT.reshape(NCT, 128, 256).transpose(1, 0, 2)),
            "wv": np.ascontiguousarray(
                wvT.reshape(NCT, 128, 128).transpose(1, 0, 2)),
            "wp": np.ascontiguousarray(
                w_proj[:, r0 : r0 + 128].T.astype(mdt)),
            "tri": tri,
        })
    return in_maps


def kernel(x, w_qkv, w_proj, b_proj, _trace=False):
    x = np.asarray(x)
    w_qkv = np.asarray(w_qkv)
    w_proj = np.asarray(w_proj)
    b_proj = np.asarray(b_proj)
    nc = _get_nc()
    in_maps = make_in_maps(x, w_qkv, w_proj)
    res = run_bass_kernel_spmd(
        nc, in_maps, core_ids=list(range(NCORES)), trace=_trace
    )
    out = np.zeros((N, C), dtype=np.float32)
    for r in res.results:
        out += r["y"].astype(np.float32)
    out += b_proj.astype(np.float32)
    out = out.reshape(B, N, C)
    if _trace:
        return out, res
    return out



# revision 37
# speedup vs baseline: 1.3585x; 1.0255x over previous
"""Causal multi-head attention (B=1, N=4096, H=16, D=64) on 8 trn2 cores.

Head-parallel tensor parallelism: 2 heads per core.  Each core reads the
full x (pre-transposed on host), computes QKV for its 2 heads, runs causal
attention in the S^T (k-partition, q-free) layout, applies its 128-column
slice of the output projection, and writes a partial [4096, 1024] y that
the host sums (plus b_proj).

Single fused pipeline: the Q/K projection of chunk j+1, the V' projection
of chunk j, and the output projection of q-block j-1 are interleaved into
the attention kt-loop of q-block j as background PE work so the tensor
engine never idles (keeping its p-state at max).  Causal masking is
multiplicative on P after exp (off the S->exp critical path), diagonal
tiles are narrowed to their valid column range, V' is produced directly by
the V projection in [seq, d] layout (no transposes), 1/l is broadcast
across partitions via a small DMA + gpsimd partition_broadcast (mid
blocks) or a K=1 PE matmul + head-split projection (final block, keeping
the tail on-chip), x/weight loads are split across engines and queues with
chunk loads deferred until one block before use, and the y writes stream
out per 128x512 tile from the idle gpsimd engine.
"""

from collections import deque

import numpy as np

import concourse.bass as bass
from concourse import bacc, masks
import concourse.tile as tile
from concourse import mybir
from concourse.bass_utils import run_bass_kernel_spmd

B, N, H, D = 1, 4096, 16, 64
C = H * D  # 1024
SCALE = D ** -0.5
NCORES = 8
HPC = H // NCORES  # heads per core = 2
F32 = mybir.dt.float32
BF16 = mybir.dt.bfloat16

NKT = N // 128      # 32 k-tiles of 128
NQB = N // 512      # 8 q-blocks of 512
NCT = C // 128      # 8 contraction tiles for the projections


DEBUG_TAPS = False


def build_nc():
    nc = bacc.Bacc("TRN2", target_bir_lowering=False)

    # xb[j, p, ct, :]: chunk-major, per-partition contiguous (8KB descriptors)
    xb = nc.dram_tensor("xb", [NQB, 128, NCT, 512], BF16,
                        kind="ExternalInput").ap()
    wqk = nc.dram_tensor("wqk", [128, NCT, 256], BF16,
                         kind="ExternalInput").ap()
    wv = nc.dram_tensor("wv", [128, NCT, 128], BF16,
                        kind="ExternalInput").ap()
    wp = nc.dram_tensor("wp", [128, C], BF16, kind="ExternalInput").ap()
    tri = nc.dram_tensor("tri", [128, 128], BF16, kind="ExternalInput").ap()
    y = nc.dram_tensor("y", [N, C], BF16, kind="ExternalOutput").ap()

    taps = None
    if DEBUG_TAPS:
        taps = {
            "dL": nc.dram_tensor("dL", [NQB, 2, 512], F32,
                                 kind="ExternalOutput").ap(),
            "dLinv": nc.dram_tensor("dLinv", [NQB, 2, 512], F32,
                                    kind="ExternalOutput").ap(),
            "dOn": nc.dram_tensor("dOn", [NQB, 128, 512], BF16,
                                  kind="ExternalOutput").ap(),
            "dQT": nc.dram_tensor("dQT", [128, N], BF16,
                                  kind="ExternalOutput").ap(),
            "dKT": nc.dram_tensor("dKT", [128, N], BF16,
                                  kind="ExternalOutput").ap(),
            "dVP": nc.dram_tensor("dVP", [128, NKT * 130], BF16,
                                  kind="ExternalOutput").ap(),
        }

    with tile.TileContext(nc) as tc:
        _body(tc, xb, wqk, wv, wp, tri, y, taps)
    nc.compile()
    return nc


def _body(tc, xb, wqk, wv, wp, tri, y, taps=None):
    nc = tc.nc
    Exp = mybir.ActivationFunctionType.Exp
    Copy = mybir.ActivationFunctionType.Copy

    persist = tc.alloc_tile_pool(name="persist", bufs=1)

    # Persistent SBUF tensors
    xt = persist.tile([128, NCT, N], BF16, tag="xt")      # full x^T, resident
    QT = persist.tile([128, N], BF16, tag="QT")           # [(h,d), n]
    KT = persist.tile([128, N], BF16, tag="KT")
    VP = persist.tile([128, NKT, 2, 65], BF16, tag="VP")  # [k, kt, h, d|1]
    Wqk = persist.tile([128, NCT, 256], BF16, tag="Wqk")
    Wv = persist.tile([128, NCT, 128], BF16, tag="Wv")
    Wp = persist.tile([128, C], BF16, tag="Wp")
    WpB = persist.tile([64, C], BF16, tag="WpB")          # Wp rows 64..127
    tri_sb = persist.tile([128, 128], BF16, tag="tri")    # [kp, qc] 1 if qc>=kp
    ones65 = persist.tile([65, 64], F32, tag="ones65")    # row 64 used as lhsT

    # startup: a single dma_start's descriptors land on ONE queue, so split
    # the critical chunk-0 / Wqk loads into several triggers spread across
    # engines (parallel queues).  Later chunks are issued an iteration ahead.
    def load_chunk(j, eng, nsplit=2):
        w = NCT // nsplit
        for i in range(nsplit):
            eng.dma_start(
                out=xt[:, w * i : w * (i + 1), 512 * j : 512 * (j + 1)],
                in_=xb[j, :, w * i : w * (i + 1), :])

    # the first trigger on each engine dispatches fastest: make them the two
    # pieces the very first matmul needs (x chunk-0 ct0 and Wqk ct0-1)
    nc.sync.dma_start(out=xt[:, 0:1, 0:512], in_=xb[0, :, 0:1, :])
    nc.scalar.dma_start(out=Wqk[:, 0:2, :], in_=wqk[:, 0:2, :])
    nc.sync.dma_start(out=Wqk[:, 2:5, :], in_=wqk[:, 2:5, :])
    nc.scalar.dma_start(out=xt[:, 1:2, 0:512], in_=xb[0, :, 1:2, :])
    nc.sync.dma_start(out=xt[:, 2:3, 0:512], in_=xb[0, :, 2:3, :])
    nc.scalar.dma_start(out=Wqk[:, 5:8, :], in_=wqk[:, 5:8, :])
    nc.scalar.dma_start(out=xt[:, 3:4, 0:512], in_=xb[0, :, 3:4, :])
    for i in range(4, 8):
        nc.gpsimd.dma_start(out=xt[:, i : i + 1, 0:512],
                            in_=xb[0, :, i : i + 1, :])
    nc.sync.dma_start(out=Wv[:, 0:4, :], in_=wv[:, 0:4, :])
    nc.scalar.dma_start(out=Wv[:, 4:8, :], in_=wv[:, 4:8, :])
    nc.sync.dma_start(out=tri_sb, in_=tri)
    load_chunk(1, nc.gpsimd, nsplit=2)
    nc.gpsimd.dma_start(out=Wp, in_=wp)
    nc.gpsimd.dma_start(out=WpB, in_=wp[64:128, :])
    nc.vector.memset(ones65, 1.0)

    # ones columns of V' (index 64 of the last axis)
    ones_st = persist.tile([128, NKT * 2], F32, tag="ones_st")
    nc.vector.memset(ones_st, 1.0)
    nc.vector.tensor_copy(
        VP.rearrange("p t g c -> p (t g) c")[:, :, 64:65],
        ones_st.rearrange("p (n o) -> p n o", o=1),
    )

    with (
        tc.tile_pool(name="spool", bufs=2, space="PSUM") as spool,   # 4 banks
        tc.tile_pool(name="opool", bufs=1, space="PSUM") as opool,   # 2 banks
        tc.tile_pool(name="smpool", bufs=2, space="PSUM") as smpool, # 2 banks
        tc.tile_pool(name="ptpool", bufs=3) as ptpool,
        tc.tile_pool(name="sbpool", bufs=2) as sbpool,
        tc.tile_pool(name="ybpool", bufs=2) as ybpool,
    ):
        def qk_closure(j):
            """Q/K projection for sequence chunk j (out [och, seq]).  Q and K
            are interleaved so consecutive matmuls accumulate into different
            PSUM banks (avoids same-bank read-modify-write bubbles)."""
            cls = []
            holder = {}

            def qk_piece(c0):
                def run():
                    if c0 == 0:
                        holder["q"] = smpool.tile([128, 512], F32, tag="sm",
                                                  name=f"psq_{j}")
                        holder["k"] = smpool.tile([128, 512], F32, tag="sm",
                                                  name=f"psk_{j}")
                    psq, psk = holder["q"], holder["k"]
                    for ct in range(c0, c0 + 2):
                        nc.tensor.matmul(
                            psq, Wqk[:, ct, 0:128],
                            xt[:, ct, 512 * j : 512 * (j + 1)],
                            start=(ct == 0), stop=(ct == NCT - 1),
                        )
                        nc.tensor.matmul(
                            psk, Wqk[:, ct, 128:256],
                            xt[:, ct, 512 * j : 512 * (j + 1)],
                            start=(ct == 0), stop=(ct == NCT - 1),
                        )
                    if c0 == NCT - 2:
                        nc.vector.tensor_copy(QT[:, 512 * j : 512 * (j + 1)],
                                              psq)
                        nc.vector.tensor_copy(KT[:, 512 * j : 512 * (j + 1)],
                                              psk)
                return run

            for c0 in range(0, NCT, 2):
                cls.append(qk_piece(c0))
            return cls

        def v_closures(j):
            """V' for chunk j directly in [seq, d] layout via N=128 matmuls
            (PE); pairs accumulate into different PSUM banks."""
            cls = []
            holder = {}

            def vchunk(s0, c0):
                def run():
                    if c0 == 0:
                        holder[f"v{s0}"] = [
                            smpool.tile([128, 512], F32, tag="sm",
                                        name=f"vps_{j}_{s0}_{i}")
                            for i in range(2)]
                    ps = holder[f"v{s0}"]
                    for ct in range(c0, c0 + 4):
                        for i in range(2):
                            kt = 4 * j + s0 + i
                            nc.tensor.matmul(
                                ps[i][:, 0:128],
                                xt[:, ct, 128 * kt : 128 * (kt + 1)],
                                Wv[:, ct, :],
                                start=(ct == 0), stop=(ct == NCT - 1),
                            )
                    if c0 == NCT - 4:
                        for i in range(2):
                            nc.vector.tensor_copy(
                                VP[:, 4 * j + s0 + i, :, 0:64],
                                ps[i][:, 0:128].rearrange(
                                    "p (g c) -> p g c", g=2),
                            )
                return run

            for s0 in (0, 2):
                for c0 in (0, 4):
                    cls.append(vchunk(s0, c0))
            return cls

        def proj_closures(qb, eps):
            """Normalize o by 1/l (PE partition-broadcast) and project."""
            cls = []
            onorm = sbpool.tile([128, 512], BF16, tag="onorm",
                                name=f"onorm_{qb}")

            def norm_mul(h):
                def run():
                    if h == 0:
                        nc.vector.tensor_mul(onorm[0:64, :],
                                             eps["osb"][0:64, 0, :],
                                             eps["linv"][:, 0, :])
                    else:
                        tmp = sbpool.tile([64, 512], BF16, tag="tmp",
                                          name=f"tmp_{qb}")
                        nc.vector.tensor_mul(tmp, eps["osb"][0:64, 1, :],
                                             eps["linv"][:, 1, :])
                        nc.sync.dma_start(out=onorm[64:128, :], in_=tmp)
                        if taps is not None:
                            nc.gpsimd.dma_start(out=taps["dL"][qb],
                                                in_=eps["osb"][64:65, :, :])
                            nc.gpsimd.dma_start(out=taps["dLinv"][qb],
                                                in_=eps["linv"][0:1, :, :])
                            nc.gpsimd.dma_start(out=taps["dOn"][qb], in_=onorm)
                return run

            cls.append(norm_mul(0))
            cls.append(norm_mul(1))

            ybuf = ybpool.tile([128, 4, C], BF16, tag="ybuf", name=f"yb_{qb}")

            def proj(s, oc):
                def run():
                    yps = smpool.tile([128, 512], F32, tag="sm")
                    nc.tensor.matmul(
                        yps, onorm[:, 128 * s : 128 * (s + 1)],
                        Wp[:, 512 * oc : 512 * (oc + 1)],
                        start=True, stop=True,
                    )
                    nc.vector.tensor_copy(ybuf[:, s, 512 * oc : 512 * (oc + 1)],
                                          yps)
                    q0 = 512 * qb + 128 * s
                    nc.gpsimd.dma_start(
                        out=y[q0 : q0 + 128, 512 * oc : 512 * (oc + 1)],
                        in_=ybuf[:, s, 512 * oc : 512 * (oc + 1)])
                return run

            for s in range(4):
                for oc in range(2):
                    cls.append(proj(s, oc))
            return cls

        # ---- fused main loop ----
        bg = deque(qk_closure(0) + v_closures(0))
        while bg:
            bg.popleft()()

        eps_prev = None
        for qb in range(NQB):
            n_kt = 4 * (qb + 1)
            q0 = 512 * qb

            if qb + 2 < NQB:
                load_chunk(qb + 2, nc.gpsimd)

            # background PE work for this block's kt loop: Q/K of the next
            # chunk (needed right at the next block's start), V' TWO chunks
            # ahead (its matmul -> evac -> xbar-transpose chain is long, so
            # give it a full block of slack), and the previous block's
            # projection.
            bg = deque()
            if qb + 1 < NQB:
                bg.extend(qk_closure(qb + 1))
                bg.extend(v_closures(qb + 1))
            if eps_prev is not None:
                bg.extend(proj_closures(qb - 1, eps_prev))

            o_ps = opool.tile([65, 2, 512], F32, tag="o", name=f"o_{qb}")
            s_tiles = {}

            def emit_S(kt, qb=qb, q0=q0, s_tiles=s_tiles):
                jr = kt - 4 * qb
                qs = 128 * jr if jr >= 0 else 0
                s_ps = spool.tile([128, 2, 512], F32, tag="s",
                                  name=f"s_{qb}_{kt}")
                for h in range(2):
                    nc.tensor.matmul(
                        s_ps[:, h, qs:512],
                        KT[64 * h : 64 * h + 64, 128 * kt : 128 * (kt + 1)],
                        QT[64 * h : 64 * h + 64, q0 + qs : q0 + 512],
                        start=True, stop=True,
                    )
                s_tiles[kt] = (s_ps, qs)

            emit_S(0)
            bg0 = len(bg)
            drained = 0
            for kt in range(n_kt):
                if kt + 1 < n_kt:
                    emit_S(kt + 1)
                s_ps, qs = s_tiles.pop(kt)
                pt = ptpool.tile([128, 2, 512], BF16, tag="pt",
                                 name=f"pt_{qb}_{kt}")
                nc.scalar.activation(pt[:, :, qs:512], s_ps[:, :, qs:512],
                                     Exp, scale=SCALE)
                if kt - 4 * qb >= 0:  # diagonal tile: zero invalid triangle
                    # one op for both heads; tri broadcast over the h dim
                    # via a stride-0 middle AP dim
                    tri_b = bass.AP(tensor=tri_sb.tensor, offset=tri_sb.offset,
                                    ap=[list(tri_sb.ap[0]), [0, 2],
                                        list(tri_sb.ap[1])])
                    nc.vector.tensor_mul(pt[:, :, qs : qs + 128],
                                         pt[:, :, qs : qs + 128], tri_b)
                for h in range(2):
                    nc.tensor.matmul(
                        o_ps[:, h, qs:512], VP[:, kt, h, :],
                        pt[:, h, qs:512],
                        start=(kt == 0), stop=(kt == n_kt - 1),
                    )
                # drain background PE work spread uniformly over ALL
                # iterations (ceil-of-remaining exhausts the queue early and
                # starves the tail iterations of filler work)
                target = -(-(kt + 1) * bg0 // n_kt)
                while drained < target and bg:
                    bg.popleft()()
                    drained += 1

            # epilogue: free o_ps via vector copy (scalar is the exp
            # bottleneck engine; gpsimd cannot read PSUM).  For the last
            # block, split so the l row (scalar) lands in parallel with the
            # d rows (vector) -- shortens the tail's serial chain.
            osb = sbpool.tile([65, 2, 512], F32, tag="osb", name=f"osb_{qb}")
            if qb == NQB - 1:
                nc.scalar.activation(osb[64:65, :, :], o_ps[64:65, :, :],
                                     Copy)
                nc.vector.tensor_copy(osb[0:64, :, :], o_ps[0:64, :, :])
            else:
                nc.vector.tensor_copy(osb, o_ps)
            if qb < NQB - 1:
                # move the l row to partition 0 (small DMA), 1/l there, then
                # gpsimd partition_broadcast across partitions 0..63 (DVE
                # lanes cannot cross partitions / need quad-aligned bases)
                l0 = sbpool.tile([1, 2, 512], F32, tag="l0", name=f"l0_{qb}")
                nc.sync.dma_start(out=l0, in_=osb[64:65, :, :])
                linv1 = sbpool.tile([1, 2, 512], F32, tag="linv1",
                                    name=f"l1_{qb}")
                nc.vector.reciprocal_approx_fast(linv1, l0)
                linv = sbpool.tile([64, 2, 512], F32, tag="linv",
                                   name=f"li_{qb}")
                nc.gpsimd.partition_broadcast(linv, linv1)
                eps_prev = {"osb": osb, "linv": linv}
            else:
                eps_prev = {"osb": osb}

        # ---- tail for the last q-block: stay on-chip (no DMA bounces).
        # Broadcast l via a K=1 PE matmul from the base-64-aligned ones
        # column, reciprocal per head, then a head-split (K=64 accumulate)
        # projection that needs no cross-partition onorm assembly.
        osb = eps_prev["osb"]
        linv7 = []
        for h in range(2):
            lbp = smpool.tile([128, 512], F32, tag="sm", name=f"lbp_{h}")
            nc.tensor.matmul(lbp[0:64, :], ones65[64:65, :],
                             osb[64:65, h, :], start=True, stop=True)
            li = sbpool.tile([64, 512], F32, tag="linv7", name=f"li7_{h}")
            nc.vector.reciprocal_approx_fast(li, lbp[0:64, :])
            linv7.append(li)
        tmp7 = []
        for h in range(2):
            t7 = sbpool.tile([64, 512], BF16, tag="tmp7", name=f"t7_{h}")
            nc.vector.tensor_mul(t7, osb[0:64, h, :], linv7[h])
            tmp7.append(t7)
        ybuf7 = ybpool.tile([128, 4, C], BF16, tag="ybuf", name="yb_7")
        q0 = 512 * (NQB - 1)
        for s in range(4):
            for oc in range(2):
                yps = smpool.tile([128, 512], F32, tag="sm",
                                  name=f"yp7_{s}_{oc}")
                nc.tensor.matmul(yps, tmp7[0][:, 128 * s : 128 * (s + 1)],
                                 Wp[0:64, 512 * oc : 512 * (oc + 1)],
                                 start=True, stop=False)
                nc.tensor.matmul(yps, tmp7[1][:, 128 * s : 128 * (s + 1)],
                                 WpB[:, 512 * oc : 512 * (oc + 1)],
                                 start=False, stop=True)
                # alternate evac engines so the copy chain halves (scalar is
                # idle in the tail), and rotate DMA issues over 3 engines
                if oc == 0:
                    nc.vector.tensor_copy(
                        ybuf7[:, s, 512 * oc : 512 * (oc + 1)], yps)
                else:
                    nc.scalar.activation(
                        ybuf7[:, s, 512 * oc : 512 * (oc + 1)], yps, Copy)
                eng = (nc.sync, nc.scalar, nc.gpsimd)[(2 * s + oc) % 3]
                eng.dma_start(
                    out=y[q0 + 128 * s : q0 + 128 * s + 128,
                          512 * oc : 512 * (oc + 1)],
                    in_=ybuf7[:, s, 512 * oc : 512 * (oc + 1)])

        if taps is not None:
            nc.gpsimd.dma_start(out=taps["dQT"], in_=QT)
            nc.gpsimd.dma_start(out=taps["dKT"], in_=KT)
            pass  # dVP tap removed (V' is now per-(kt,h) tiles)

    persist.release()


_NC_CACHE = {}


def _get_nc():
    if "nc" not in _NC_CACHE:
        _NC_CACHE["nc"] = build_nc()
    return _NC_CACHE["nc"]


def make_in_maps(x, w_qkv, w_proj):
    """Host-side sharding: per-core input dicts."""
    from concourse import mybir as _mb
    mdt = _mb.dt.np(BF16)
    xTh = x[0].T.astype(mdt)  # [C, N]
    # chunk-major, per-partition contiguous: xb[j, p, ct, n]
    xbh = np.ascontiguousarray(
        xTh.reshape(NCT, 128, NQB, 512).transpose(2, 1, 0, 3))
    tri = (np.arange(128)[None, :] >= np.arange(128)[:, None]).astype(mdt)
    in_maps = []
    for m in range(NCORES):
        r0 = HPC * D * m  # 128*m
        wq = w_qkv[r0 : r0 + 128]
        wk = w_qkv[C + r0 : C + r0 + 128]
        wvm = w_qkv[2 * C + r0 : 2 * C + r0 + 128]
        wqkT = np.concatenate([wq, wk], 0).T.astype(mdt)  # [C, 256]
        wvT = wvm.T.astype(mdt)                           # [C, 128]
        in_maps.append({
            "xb": xbh,
            "wqk": np.ascontiguousarray(
                wqkT.reshape(NCT, 128, 256).transpose(1, 0, 2)),
            "wv": np.ascontiguousarray(
                wvT.reshape(NCT, 128, 128).transpose(1, 0, 2)),
            "wp": np.ascontiguousarray(
                w_proj[:, r0 : r0 + 128].T.astype(mdt)),
            "tri": tri,
        })
    return in_maps


def kernel(x, w_qkv, w_proj, b_proj, _trace=False):
    x = np.asarray(x)
    w_qkv = np.asarray(w_qkv)
    w_proj = np.asarray(w_proj)
    b_proj = np.asarray(b_proj)
    nc = _get_nc()
    in_maps = make_in_maps(x, w_qkv, w_proj)
    res = run_bass_kernel_spmd(
        nc, in_maps, core_ids=list(range(NCORES)), trace=_trace
    )
    out = np.zeros((N, C), dtype=np.float32)
    for r in res.results:
        out += r["y"].astype(np.float32)
    out += b_proj.astype(np.float32)
    out = out.reshape(B, N, C)
    if _trace:
        return out, res
    return out

